# revision 2
# baseline (speedup 1.0000x reference)
"""LoRA-injected linear layer on 8 Trainium2 NeuronCores.

Computes y = x @ (W + down @ up)^T + bias for
  x [4, 2048, 4096] f32, W [4096, 4096] f32, down [4096, 16], up [16, 4096].

Sharding: 2 token-groups x 4 out-feature-groups = 8 cores.
Each core computes y_shard [4096 tokens, 1024 out features]:
  - builds W_eff^T[:, shard] = W^T + up^T @ down^T on-chip (PE K=16 matmuls
    + DVE adds), kept SBUF-resident (16.8 MB),
  - streams x^T token tiles and runs 32x2x32 accumulating bf16 matmuls
    (fp32 PSUM accumulate -> ~4e-3 rel err, full PE rate + FWL weight loads),
  - fuses the bias add into the PSUM->SBUF drain.

Host side does layout-only transforms (transposes/tiling) so all DMAs have
>=512B contiguous runs.
"""

import numpy as np

import concourse.bass as bass
import concourse.bacc as bacc
import concourse.mybir as mybir
import concourse.tile as tile
from concourse.bass_utils import run_bass_kernel_spmd

# Problem dims (hardcoded per contract).
B, S, IN, OUT, R = 4, 2048, 4096, 4096, 16
NCORES = 8
TG, OG = 2, 4          # token groups x out-feature groups
T = B * S              # 8192 total tokens
TC = T // TG           # 4096 tokens per core
OC = OUT // OG         # 1024 out features per core
P = 128                # partition dim
NT = TC // P           # 32 token tiles per core
NI = IN // P           # 32 contraction tiles
OB = 512               # PSUM-bank-wide output block
NOB = OC // OB         # 2 output blocks per core

F32 = mybir.dt.float32
BF16 = mybir.dt.bfloat16

_CACHE = {}


def _build_nc():
    nc = bacc.Bacc(None, target_bir_lowering=False)

    # DRAM I/O (per-core shards; same program on all 8 cores).
    # xts[tt, it, i, t] = x^T[it*128+i, tt*128+t] for this core's token group.
    xts_d = nc.declare_dram_parameter("xts", [NT // 2, NI, P, 2 * P], BF16, isOutput=False)
    wt_d = nc.declare_dram_parameter("wt", [IN, OC], BF16, isOutput=False)
    upt_d = nc.declare_dram_parameter("upt", [NI, R, P], BF16, isOutput=False)
    dnt_d = nc.declare_dram_parameter("dnt", [R, OC], BF16, isOutput=False)
    bias_d = nc.declare_dram_parameter("biasb", [P, OC], F32, isOutput=False)
    y_d = nc.declare_dram_parameter("y", [TC, OC], F32, isOutput=True)

    with tile.TileContext(nc) as tc:
        with (
            tc.tile_pool(name="weff", bufs=1) as weff_pool,
            tc.tile_pool(name="const", bufs=1) as const_pool,
            tc.tile_pool(name="io", bufs=2) as io_pool,
            tc.tile_pool(name="psum", bufs=2, space="PSUM") as psum_pool,
        ):
            dnt_sb = const_pool.tile([R, OC], BF16, name="dnt_sb")
            nc.sync.dma_start(out=dnt_sb[:], in_=dnt_d[:])
            bias_sb = const_pool.tile([P, OC], F32, name="bias_sb")
            nc.sync.dma_start(out=bias_sb[:], in_=bias_d[:])

            # Phase 1: W_eff^T tiles, one [128, OC] tile per contraction tile,
            # resident for the whole kernel.
            weff = []
            for i in range(NI):
                wt_t = weff_pool.tile(
                    [P, OC], BF16, name=f"weff{i}", tag=f"weff{i}", bufs=1
                )
                nc.sync.dma_start(out=wt_t[:], in_=wt_d[i * P : (i + 1) * P, :])
                up_t = io_pool.tile([R, P], BF16, name="up_t", tag="up_t", bufs=2)
                nc.sync.dma_start(out=up_t[:], in_=upt_d[i])
                for ob in range(NOB):
                    osl = slice(ob * OB, (ob + 1) * OB)
                    lps = psum_pool.tile(
                        [P, OB], F32, name="lps", tag="lps", bufs=2
                    )
                    nc.tensor.matmul(
                        lps[:],
                        lhsT=up_t[:],
                        rhs=dnt_sb[:, osl],
                        start=True,
                        stop=True,
                    )
                    nc.vector.tensor_add(
                        out=wt_t[:, osl], in0=wt_t[:, osl], in1=lps[:]
                    )
                weff.append(wt_t)

            # Phase 2: main matmul, token tiles in pairs (512B DMA runs).
            for tt2 in range(NT // 2):
                xts_t = io_pool.tile(
                    [P, NI, 2 * P], BF16, name="xts_t", tag="xts_t", bufs=2
                )
                nc.sync.dma_start(
                    out=xts_t[:], in_=xts_d[tt2].transpose([1, 0, 2])
                )
                for sub in range(2):
                    tsl = slice(sub * P, (sub + 1) * P)
                    y_sb = io_pool.tile(
                        [P, OC], F32, name="y_sb", tag="y_sb", bufs=3
                    )
                    for ob in range(NOB):
                        osl = slice(ob * OB, (ob + 1) * OB)
                        ps = psum_pool.tile(
                            [P, OB], F32, name="ps", tag="ps", bufs=4
                        )
                        for it in range(NI):
                            nc.tensor.matmul(
                                ps[:],
                                lhsT=xts_t[:, it, tsl],
                                rhs=weff[it][:, osl],
                                start=(it == 0),
                                stop=(it == NI - 1),
                            )
                        nc.vector.tensor_add(
                            out=y_sb[:, osl], in0=ps[:], in1=bias_sb[:, osl]
                        )
                    nc.sync.dma_start(
                        out=y_d[(tt2 * 2 + sub) * P : (tt2 * 2 + sub + 1) * P, :],
                        in_=y_sb[:],
                    )

    nc.compile()
    return nc


def _shard_inputs(x, old_weight, old_bias, lora_down, lora_up):
    import ml_dtypes

    bf16 = np.dtype(ml_dtypes.bfloat16)
    x2 = np.ascontiguousarray(x, dtype=np.float32).reshape(T, IN).astype(bf16)
    wtf = np.ascontiguousarray(old_weight.T).astype(bf16)   # [IN, OUT]
    dnf = np.ascontiguousarray(lora_down.T).astype(bf16)    # [R, OUT]
    # upt[it, r, p] = lora_up[r, it*128+p]
    upt = np.ascontiguousarray(
        np.asarray(lora_up, np.float32).astype(bf16).reshape(R, NI, P)
        .transpose(1, 0, 2)
    )
    in_maps = []
    for c in range(NCORES):
        g, j = divmod(c, OG)
        xs = x2[g * TC : (g + 1) * TC]                # [TC, IN] bf16
        # xts[tt2, it, i, u] = xs[tt2*256+u, it*128+i]
        xts = np.ascontiguousarray(
            xs.reshape(NT // 2, 2 * P, NI, P).transpose(0, 2, 3, 1)
        )
        osl = slice(j * OC, (j + 1) * OC)
        in_maps.append(
            {
                "xts": xts,
                "wt": np.ascontiguousarray(wtf[:, osl]),
                "upt": upt,
                "dnt": np.ascontiguousarray(dnf[:, osl]),
                "biasb": np.ascontiguousarray(
                    np.broadcast_to(
                        np.asarray(old_bias, np.float32)[osl], (P, OC)
                    )
                ),
            }
        )
    return in_maps


def _get_nc():
    if "nc" not in _CACHE:
        _CACHE["nc"] = _build_nc()
    return _CACHE["nc"]


def _unshard(results):
    y = np.empty((T, OUT), dtype=np.float32)
    for c in range(NCORES):
        g, j = divmod(c, OG)
        y[g * TC : (g + 1) * TC, j * OC : (j + 1) * OC] = results[c]["y"]
    return y.reshape(B, S, OUT)


def _run(inputs, trace=False, trace_cores=None):
    nc = _get_nc()
    in_maps = _shard_inputs(**inputs)
    res = run_bass_kernel_spmd(
        nc,
        in_maps,
        list(range(NCORES)),
        trace=trace,
        trace_cores=trace_cores,
    )
    return _unshard(res.results), res


def kernel(**inputs):
    y, _ = _run(inputs)
    return y

